# revision 27
# baseline (speedup 1.0000x reference)
"""Trainium2 Bass kernel for the 3-layer spiking neural network (DSNN).

Strategy
--------
Data-parallel over batch: 256 rows / 8 cores = 32 per core, weights
replicated, zero collectives. Inside each core:

  - Layer 2 has no reset, so mem2 = (sum_t w_t out1(t)) @ W2 exactly
    (closed-form alpha/beta decay weights). The sum is split: blocks
    0..NB-3 accumulate into `abar` (final one iteration before the
    drain, so its 8 matmuls overlap the drain steps and run warm) and
    the last two blocks into `abar2` (one 8-matmul PSUM-accumulated
    pass at the very end).
  - The layer-1 synapse recurrence is folded into the matmul operand:
    mm1's moving tensor is the spike TRACE strace_t = a*strace_{t-1}
    + s0_t, so y1_t = strace_t @ W1 exactly (linearity).
  - Timestep-blocked matmuls (Tb=16 -> 512 moving columns), weights /
    spikes / trace all fp16. The PE streams at 1 col/cycle (2.0 GHz
    under the sustained-load P0 power state) and is the roofline:
    ~96 N=512 matmuls per 16-step block, LDWEIGHTS fully hidden.
  - ALL recurrent state is fp16 (membranes, drive slots, trace, abar)
    so the per-step DVE ops run in the hardware's 2X_1PORT perf mode
    via HAND-WRITTEN dual-lane uop programs (the stock custom-DVE
    lowering only emits 1x): SNN_RESETX does both layers' membrane
    decay+reset in one [128,512] op (~420ns vs ~828 at 1x/f32) using
    the select-free form out = x*(x >= -thr), x = beta*negm - h
    (0.0 encodes "spiked"); SNN_TRACEX/SNN_ABARX update the trace EMA
    and the w_t-weighted spike sum on the slot halves (~290ns each).
    The 2x programs are installed by seeding dve_ops._COMPILE_CACHE
    with a DveOpSpec carrying uops_2x + perf_max=1 and setting
    perf_max on each emitted instruction. Host-side bit-accurate
    simulation predicted rel-l2 1.3326e-2 (HW: 1.3320e-2) vs the
    2e-2 gate.
  - Layer-1 trails layer-0 by two blocks so its drive h1 = trace @ W1
    is ready; membrane state ping-pongs between nmA/nmB.
  - HAM keep-warm: junk matmuls bridge the input-DMA fill (the PE
    clock-gate un-throttles before the first real matmul) and pinned
    dummy matmuls ride the DVE-only drain steps so the final W2
    matmuls run at full clock. W0h is DMA'd first (it gates mm0) in 4
    chunk transfers; each rt block is one DMA trigger.

Per main step the DVE runs three 2x ops: RESETX [128,512] + TRACEX +
ABARX [128,256] each (~1.0us/step vs 2.65us in the original kernel).
Steady state is PE-bound at ~1.55us/step. Measured end-to-end
(traced): 163-167us, from 225.3us at session start and 273.6us for
the original unblocked kernel.
"""

import numpy as np

ALPHA = 0.9
BETA = 0.85
THR = 1.0
T = 99            # timesteps actually simulated (t = 1..99 of 100)
BCORE = 32        # batch per core
NCORES = 8
TBM = 16          # main block size (Nk = 512 moving cols)
BLOCK_SIZES = [12, 16, 16, 16, 16, 16, 4, 3]
assert sum(BLOCK_SIZES) == T
NB = len(BLOCK_SIZES)
TSTART = [sum(BLOCK_SIZES[:i]) for i in range(NB)]

_CACHE = {}


def _register_custom_ops():
    """SNN_RESET: m = in0*s0 + in1; out = m>s1 ? 0 : -m   (negated membrane;
    0.0 encodes "spiked").
    SNN_TRACE: out = in1*s0 + (in0 == 0)                  (spike trace EMA).
    SNN_ABAR:  out = (in0 == 0) ? in1 + s0 : in1          (weighted spikes).
    """
    import concourse.dve_ops as dve_ops
    if "SNN_RESET" in dve_ops._SUB_OPCODE_FOR_NAME:
        return (next(o for o in dve_ops.OPS if o.name == "SNN_RESETX"),
                next(o for o in dve_ops.OPS if o.name == "SNN_TRACEX"),
                next(o for o in dve_ops.OPS if o.name == "SNN_ABARX"),
                next(o for o in dve_ops.OPS if o.name == "SNN_TA"))
    from concourse.dve_spec import (
        Spec, Src0, Src1, Zero, select, eq, lower, _has_src1, SubIdx)
    from concourse.dve_uop import DveOpSpec

    def make(name, spec, subdim=False):
        row = dve_ops._CUSTOM_DVE_ROW_BASE + len(dve_ops.OPS)
        assert row < 0x20
        dve_ops._SUB_OPCODE_FOR_NAME[name] = row
        shas = {}
        for ver in ("v3", "v4"):
            uops = lower(spec, ver=ver)
            shas[ver] = DveOpSpec(name=name, opcode=row, uops=uops,
                                  rd1_en=_has_src1(spec)).sha(ver)
        op = dve_ops.DveOp(name, spec, subdim=subdim, uops_sha=shas)
        dve_ops.OPS.append(op)
        dve_ops.CUSTOM_DVE_SPECS[name] = spec
        return op

    from concourse.dve_spec import C0, C1
    f32 = np.float32
    _m = Src0 * C0 + Src1
    reset = make("SNN_RESET", Spec(
        body=select(_m > C1, Zero, Zero - _m),
        reference=lambda in0, in1, s0, s1, imm2:
            np.where((in0 * f32(s0) + in1) > f32(s1),
                     f32(0.0), -(in0 * f32(s0) + in1)).astype(f32),
    ))
    trace = make("SNN_TRACE", Spec(
        body=Src1 * C0 + eq(Src0, Zero),
        reference=lambda in0, in1, s0, s1, imm2:
            (in1 * f32(s0) + (in0 == 0.0)).astype(f32),
    ))
    abar_op = make("SNN_ABAR", Spec(
        body=select(eq(Src0, Zero), Src1 + C0, Src1),
        reference=lambda in0, in1, s0, s1, imm2:
            np.where(in0 == 0.0, in1 + f32(s0), in1).astype(f32),
    ))

    def _ta_ref(in0, in1, s0, s1, imm2):
        # in0/in1/out: [P, 2, N] pages; page 0 = trace, page 1 = abar
        sp = (in0 == 0.0).astype(f32)
        out = np.empty_like(in1, dtype=f32)
        out[:, 0] = in1[:, 0] * f32(s0) + sp[:, 0]
        out[:, 1] = np.where(in0[:, 1] == 0.0, in1[:, 1] + f32(s1), in1[:, 1])
        return out

    _sp = eq(Src0, Zero)
    ta_op = make("SNN_TA", Spec(
        body=select(SubIdx, Src1 + _sp * C1, Src1 * C0 + _sp),
        reference=_ta_ref,
    ), subdim=True)

    # ---- SNN_RESETX: fp16 membrane update with a hand-built 2X_1PORT
    # program. Same semantics as SNN_RESET but on the negated form
    # x = beta*negm - h (so s0=+beta, s1=-thr):
    #   out = x * (x >= -thr)   (spiked -> exact 0.0)
    # With all-fp16 step-1 operands and perf_max=1 the engine runs the 2x
    # program: 2 elems/cycle, halving the dominant per-step DVE cost.
    from concourse.dve_uop import (
        UopConfig, InpSel, AluInp, DelayInp, OutSel, OutPath,
        AluOp as UAluOp, Trigger, ENABLE)

    _x = Src0 * C0 - Src1
    rx_spec = Spec(
        body=_x * (_x >= C1),
        reference=lambda in0, in1, s0, s1, imm2: (
            lambda xv: np.where(xv >= f32(s1), xv, f32(0.0)).astype(f32)
        )(in0 * f32(s0) - in1),
    )
    rx_row = dve_ops._CUSTOM_DVE_ROW_BASE + len(dve_ops.OPS)
    assert rx_row < 0x20
    dve_ops._SUB_OPCODE_FOR_NAME["SNN_RESETX"] = rx_row
    rx_1x = lower(rx_spec, ver="v3")
    assert len(rx_1x) == 1

    # lanes: 0=SRC_0(negm lo) 1=SRC_1(h lo) 2=SRC_0_HI 3=SRC_1_HI
    #        4=C0(beta) 5=C1(-thr); chains at b0: d[c] <- lane c+1
    u = UopConfig()
    u.enable_input(InpSel.SRC_0, 0)
    u.enable_input(InpSel.SRC_1, 1)
    u.enable_input(InpSel.SRC_0_HI, 2)
    u.enable_input(InpSel.SRC_1_HI, 3)
    u.enable_input(InpSel.CONST_0, 4)
    u.enable_input(InpSel.CONST_1, 5)
    u.require_inp0 = ENABLE
    u.require_inp1 = ENABLE
    u.trigger = (Trigger.SRC_TENSOR_DONE, Trigger.NONE, Trigger.NONE)
    u.enable_output(OutSel.DELAY_0, OutPath.WR0_LO)
    u.enable_output(OutSel.ALU_OUT, OutPath.WR0_HI)
    b = u.datapath_config
    b[0].enable_alu(UAluOp.MULTIPLY, AluInp.PREV_ALU_OUT, AluInp.PREV_DELAY_3)
    for c in range(5):
        b[0].enable_delay_from_src(DelayInp.PREV_DELAY, c)
    b[1].enable_alu(UAluOp.SUBTRACT, AluInp.PREV_ALU_OUT, AluInp.PREV_DELAY_0)
    for c in (1, 2, 3, 4):
        b[1].pass_through_delay(c)
    b[2].enable_alu(UAluOp.MULTIPLY, AluInp.PREV_DELAY_1, AluInp.PREV_DELAY_3)
    b[2].enable_delay_from_src(DelayInp.PREV_ALU_OUT, 0)
    for c in (2, 4):
        b[2].pass_through_delay(c)
    b[3].enable_alu(UAluOp.SUBTRACT, AluInp.PREV_ALU_OUT, AluInp.PREV_DELAY_2)
    for c in (0, 4):
        b[3].pass_through_delay(c)
    b[4].enable_alu(UAluOp.IS_GE, AluInp.PREV_DELAY_0, AluInp.PREV_DELAY_4)
    b[4].enable_delay_from_src(DelayInp.PREV_ALU_OUT, 1)
    for c in (0, 4):
        b[4].pass_through_delay(c)
    b[5].enable_alu(UAluOp.MULTIPLY, AluInp.PREV_ALU_OUT, AluInp.PREV_DELAY_0)
    for c in (1, 4):
        b[5].pass_through_delay(c)
    b[6].enable_alu(UAluOp.IS_GE, AluInp.PREV_DELAY_1, AluInp.PREV_DELAY_4)
    b[6].enable_delay_from_src(DelayInp.PREV_ALU_OUT, 0)
    b[6].pass_through_delay(1)
    b[7].enable_alu(UAluOp.MULTIPLY, AluInp.PREV_ALU_OUT, AluInp.PREV_DELAY_1)
    b[7].pass_through_delay(0)
    u.validate("v3")

    rx_spec2 = DveOpSpec(name="SNN_RESETX", opcode=rx_row, uops=rx_1x,
                         uops_2x=[u], perf_max=1, rd1_en=True)
    dve_ops._COMPILE_CACHE[("SNN_RESETX", "v3")] = rx_spec2
    rx_shas = {"v3": rx_spec2.sha("v3"),
               "v4": DveOpSpec(name="SNN_RESETX", opcode=rx_row,
                               uops=lower(rx_spec, ver="v4"),
                               rd1_en=True).sha("v4")}
    resetx = dve_ops.DveOp("SNN_RESETX", rx_spec, subdim=False,
                           uops_sha=rx_shas)
    dve_ops.OPS.append(resetx)
    dve_ops.CUSTOM_DVE_SPECS["SNN_RESETX"] = rx_spec

    # ---- SNN_TRACEX / SNN_ABARX: 2x variants of the trace-EMA and the
    # weighted-spike accumulation. Both bodies are 3 ALU ops per lane, so
    # the duplicated (lo|hi) chain fits in 6 of the 8 datapath blocks.
    def _make_2x(name, spec, wire):
        row2 = dve_ops._CUSTOM_DVE_ROW_BASE + len(dve_ops.OPS)
        assert row2 < 0x20
        dve_ops._SUB_OPCODE_FOR_NAME[name] = row2
        u1x = lower(spec, ver="v3")
        assert len(u1x) == 1
        u2 = UopConfig()
        u2.enable_input(InpSel.SRC_0, 0)
        u2.enable_input(InpSel.SRC_1, 1)
        u2.enable_input(InpSel.SRC_0_HI, 2)
        u2.enable_input(InpSel.SRC_1_HI, 3)
        u2.enable_input(InpSel.CONST_0, 4)
        u2.enable_input(InpSel.ZERO, 5)
        u2.require_inp0 = ENABLE
        u2.require_inp1 = ENABLE
        u2.trigger = (Trigger.SRC_TENSOR_DONE, Trigger.NONE, Trigger.NONE)
        u2.enable_output(OutSel.DELAY_0, OutPath.WR0_LO)
        u2.enable_output(OutSel.ALU_OUT, OutPath.WR0_HI)
        wire(u2.datapath_config)
        u2.validate("v3")
        spec2 = DveOpSpec(name=name, opcode=row2, uops=u1x, uops_2x=[u2],
                          perf_max=1, rd1_en=True)
        dve_ops._COMPILE_CACHE[(name, "v3")] = spec2
        shas2 = {"v3": spec2.sha("v3"),
                 "v4": DveOpSpec(name=name, opcode=row2,
                                 uops=lower(spec, ver="v4"),
                                 rd1_en=True).sha("v4")}
        op2 = dve_ops.DveOp(name, spec, subdim=False, uops_sha=shas2)
        dve_ops.OPS.append(op2)
        dve_ops.CUSTOM_DVE_SPECS[name] = spec
        return op2

    # lanes: 0=SRC_0 1=SRC_1 2=SRC_0_HI 3=SRC_1_HI 4=C0 5=ZERO
    # chains at b0: d0<-L1 d1<-L2 d2<-L3 d3<-L4 d4<-L5
    def _wire_trace(b):
        # out = Src1*C0 + (Src0 == 0)
        b[0].enable_alu(UAluOp.IS_EQ, AluInp.PREV_ALU_OUT, AluInp.PREV_DELAY_4)
        for c in range(5):
            b[0].enable_delay_from_src(DelayInp.PREV_DELAY, c)
        b[1].enable_alu(UAluOp.MULTIPLY, AluInp.PREV_DELAY_0, AluInp.PREV_DELAY_3)
        b[1].enable_delay_from_src(DelayInp.PREV_ALU_OUT, 0)
        for c in (1, 2, 3, 4):
            b[1].pass_through_delay(c)
        b[2].enable_alu(UAluOp.ADD, AluInp.PREV_ALU_OUT, AluInp.PREV_DELAY_0)
        for c in (1, 2, 3, 4):
            b[2].pass_through_delay(c)
        b[3].enable_alu(UAluOp.IS_EQ, AluInp.PREV_DELAY_1, AluInp.PREV_DELAY_4)
        b[3].enable_delay_from_src(DelayInp.PREV_ALU_OUT, 0)
        for c in (2, 3):
            b[3].pass_through_delay(c)
        b[4].enable_alu(UAluOp.MULTIPLY, AluInp.PREV_DELAY_2, AluInp.PREV_DELAY_3)
        b[4].enable_delay_from_src(DelayInp.PREV_ALU_OUT, 1)
        b[4].pass_through_delay(0)
        b[5].enable_alu(UAluOp.ADD, AluInp.PREV_ALU_OUT, AluInp.PREV_DELAY_1)
        b[5].pass_through_delay(0)
        b[6].pass_through_alu()
        b[6].pass_through_delay(0)
        b[7].pass_through_alu()
        b[7].pass_through_delay(0)

    def _wire_abar(b):
        # out = Src1 + (Src0 == 0)*C0
        b[0].enable_alu(UAluOp.IS_EQ, AluInp.PREV_ALU_OUT, AluInp.PREV_DELAY_4)
        for c in range(5):
            b[0].enable_delay_from_src(DelayInp.PREV_DELAY, c)
        b[1].enable_alu(UAluOp.MULTIPLY, AluInp.PREV_ALU_OUT, AluInp.PREV_DELAY_3)
        for c in (0, 1, 2, 3, 4):
            b[1].pass_through_delay(c)
        b[2].enable_alu(UAluOp.ADD, AluInp.PREV_ALU_OUT, AluInp.PREV_DELAY_0)
        for c in (1, 2, 3, 4):
            b[2].pass_through_delay(c)
        b[3].enable_alu(UAluOp.IS_EQ, AluInp.PREV_DELAY_1, AluInp.PREV_DELAY_4)
        b[3].enable_delay_from_src(DelayInp.PREV_ALU_OUT, 0)
        for c in (2, 3):
            b[3].pass_through_delay(c)
        b[4].enable_alu(UAluOp.MULTIPLY, AluInp.PREV_ALU_OUT, AluInp.PREV_DELAY_3)
        for c in (0, 2):
            b[4].pass_through_delay(c)
        b[5].enable_alu(UAluOp.ADD, AluInp.PREV_ALU_OUT, AluInp.PREV_DELAY_2)
        b[5].pass_through_delay(0)
        b[6].pass_through_alu()
        b[6].pass_through_delay(0)
        b[7].pass_through_alu()
        b[7].pass_through_delay(0)

    tracex = _make_2x("SNN_TRACEX", Spec(
        body=Src1 * C0 + eq(Src0, Zero),
        reference=lambda in0, in1, s0, s1, imm2:
            (in1 * f32(s0) + (in0 == 0.0)).astype(f32)), _wire_trace)
    abarx = _make_2x("SNN_ABARX", Spec(
        body=select(eq(Src0, Zero), Src1 + C0, Src1),
        reference=lambda in0, in1, s0, s1, imm2:
            np.where(in0 == 0.0, in1 + f32(s0), in1).astype(f32)), _wire_abar)

    return resetx, tracex, abarx, ta_op


def _round_m11(x):
    # hw float32r = e8m11, round-to-nearest on the 12 dropped bits
    xi = np.ascontiguousarray(np.asarray(x, np.float32)).view(np.uint32).astype(np.uint64)
    bias = np.uint64(0x7FF) + ((xi >> np.uint64(12)) & np.uint64(1))
    return ((xi + bias) & np.uint64(0xFFFFF000)).astype(np.uint32).view(np.float32)


def _decay_weights():
    # w_j = sum_{k=0}^{T-1-j} BETA^(T-1-j-k) * ALPHA^k
    w = np.zeros(T, np.float64)
    for j in range(T):
        n = T - 1 - j
        k = np.arange(n + 1)
        w[j] = np.sum(BETA ** (n - k) * (ALPHA ** k))
    return w.astype(np.float32)


def build_program():
    if "nc" in _CACHE:
        return _CACHE["nc"]
    import concourse.bacc as bacc
    import concourse.mybir as mybir
    import concourse.tile as tile

    f32 = mybir.dt.float32
    f32r = mybir.dt.float32r
    f16 = mybir.dt.float16
    A = mybir.AluOpType
    Act = mybir.ActivationFunctionType

    OP_RESET, OP_TRACE, OP_ABAR, OP_TA = _register_custom_ops()
    W = _decay_weights()

    nc = bacc.Bacc("TRN2", target_bir_lowering=False, debug=False,
                   enable_asserts=False, num_devices=NCORES)

    RT = nc.dram_tensor("RT", [512, T * BCORE], f32, kind="ExternalInput").ap()
    xT = nc.dram_tensor("xT", [512, BCORE], f32, kind="ExternalInput").ap()
    W0h = nc.dram_tensor("W0h", [512, 1024], f16, kind="ExternalInput").ap()
    W1d = nc.dram_tensor("W1d", [1024, 1024], f16, kind="ExternalInput").ap()
    W2d = nc.dram_tensor("W2d", [1024, 512], f16, kind="ExternalInput").ap()
    b0d = nc.dram_tensor("b0d", [128, 8], f32, kind="ExternalInput").ap()
    outd = nc.dram_tensor("out", [BCORE, 512], f32, kind="ExternalOutput").ap()

    with tile.TileContext(nc) as tc:
        with (
            tc.tile_pool(name="const", bufs=1) as cpool,
            tc.tile_pool(name="rt", bufs=3) as rt_pool,
            tc.tile_pool(name="sblk", bufs=2) as s_pool,
            tc.tile_pool(name="s0p", bufs=2) as s0_pool,
            tc.tile_pool(name="drv", bufs=2) as drv_pool,
            tc.tile_pool(name="ps", bufs=3, space="PSUM") as ps_pool,
            tc.tile_pool(name="warm", bufs=1, space="PSUM") as warm_pool,
        ):
            # ---- constants ----
            w0h_sb = cpool.tile([128, 4 * 1024], f16, tag="w0h")
            w1_sb = cpool.tile([128, 8 * 1024], f16, tag="w1")
            b0_sb = cpool.tile([128, 8], f32, tag="b0")
            xt_sb = cpool.tile([128, 4 * BCORE], f32, tag="xt")

            # PE warm-up fodder: junk operands with no DMA deps, plus a
            # stable fp32 tile the drain keep-warm matmuls stream from.
            junk = cpool.tile([128, 384], f16, tag="junk")
            junkf = cpool.tile([128, 256], f16, tag="junkf")
            nc.gpsimd.memset(junk[:], 0.0)
            nc.gpsimd.memset(junkf[:], 0.0)
            ps_w = warm_pool.tile([128, 256], f32, tag="psw")

            # ---- state ----
            # negm ping-pong: [0:256) = layer-0 negm (c,b), [256:512) = layer-1
            nmA = cpool.tile([128, 512], f16, tag="nmA")
            nmB = cpool.tile([128, 512], f16, tag="nmB")
            abar = cpool.tile([128, 256], f16, tag="abar")
            abar2 = cpool.tile([128, 256], f16, tag="abar2")
            drv9 = cpool.tile([128, BLOCK_SIZES[NB - 1] * 512], f16,
                              tag="drv9")
            for st in (nmA, nmB, abar, abar2):
                nc.vector.memset(st[:], 0.0)
            nm = [nmA, nmB]
            gstep = [0]

            rt4 = RT.rearrange("(c p) n -> p c n", p=128)
            rt_t, sblk_t, s0_t, drv_t = {}, {}, {}, {}

            def stage_dma_rt(k):
                Tb = BLOCK_SIZES[k]
                Nk = Tb * BCORE
                rt = rt_pool.tile([128, 4 * TBM * BCORE], f32, tag="rt")
                nc.sync.dma_start(
                    out=rt[:, :4 * Nk].rearrange("p (c n) -> p c n", c=4),
                    in_=rt4[:, :, TSTART[k] * BCORE: TSTART[k] * BCORE + Nk])
                rt_t[k] = rt

            def stage_sg(k):
                # spike-gen: compare x (broadcast over t) against rt.
                # Output dtype matches the W0 flavor mm0 will use:
                # f32r for early blocks, fp16 for the small late blocks.
                Tb = BLOCK_SIZES[k]
                Nk = Tb * BCORE
                rt = rt_t.pop(k)
                sblk = s_pool.tile([128, 4 * TBM * BCORE], f16, tag="sblk")
                xc = (xt_sb[:].rearrange("p (c b) -> p c b", c=4)
                      .unsqueeze(2).broadcast_to([128, 4, Tb, BCORE]))
                ssl = sblk[:, :4 * Nk].rearrange("p (c t b) -> p c t b", c=4, t=Tb)
                rsl = rt[:, :4 * Nk].rearrange("p (c t b) -> p c t b", c=4, t=Tb)
                if k == 0:
                    # chunked so the first compares pipeline with the DMA
                    for c in range(4):
                        nc.vector.tensor_tensor(
                            out=ssl[:, c:c + 1], in0=xc[:, c:c + 1],
                            in1=rsl[:, c:c + 1], op=A.is_gt)
                else:
                    nc.vector.tensor_tensor(out=ssl, in0=xc, in1=rsl, op=A.is_gt)
                sblk_t[k] = sblk

            def stage_mm0(k):
                # H0 = S @ W0 -> drive tile k, slot lanes [0:256), t-major
                Tb = BLOCK_SIZES[k]
                Nk = Tb * BCORE
                sblk = sblk_t.pop(k)
                w0t = w0h_sb
                drv = drv_t[k]
                dv = drv[:].rearrange("p (t l) -> p t l", t=TBM)
                for c in range(8):
                    ps = ps_pool.tile([128, TBM * BCORE], f32, tag="ps")
                    for ki in range(4):
                        nc.tensor.matmul(
                            ps[:, :Nk],
                            lhsT=w0t[:, ki * 1024 + c * 128: ki * 1024 + (c + 1) * 128],
                            rhs=sblk[:, ki * Nk:(ki + 1) * Nk],
                            start=(ki == 0), stop=(ki == 3))
                    # PSUM (t,b) -> drive slots, bias fold
                    nc.scalar.activation(
                        out=dv[:, 0:Tb, c * BCORE:(c + 1) * BCORE],
                        in_=ps[:, :Nk].rearrange("p (t b) -> p t b", t=Tb),
                        func=Act.Identity, bias=b0_sb[:, c:c + 1], scale=1.0)

            def stage_mm1(k, out_drv=None):
                # H1 = strace @ W1 -> drive tile k+2, slot lanes [256:512)
                Tb = BLOCK_SIZES[k]
                Nk = Tb * BCORE
                s0blk = s0_t[k]
                # slots are 512 wide: lanes [0:256) trace, [256:512) abar
                s0v = s0blk[:, :Tb * 512].rearrange("p (t l) -> p t l", t=Tb)
                if out_drv is not None:
                    drv, tdim = out_drv, Tb
                else:
                    drv, tdim = drv_t[k + 2], TBM
                dv = drv[:].rearrange("p (t l) -> p t l", t=tdim)
                for c in range(8):
                    ps = ps_pool.tile([128, TBM * BCORE], f32, tag="ps")
                    for ki in range(8):
                        nc.tensor.matmul(
                            ps[:, :Nk],
                            lhsT=w1_sb[:, ki * 1024 + c * 128: ki * 1024 + (c + 1) * 128],
                            rhs=s0v[:, :, ki * BCORE:(ki + 1) * BCORE],
                            start=(ki == 0), stop=(ki == 7))
                    nc.scalar.activation(
                        out=dv[:, 0:Tb, 256 + c * BCORE:256 + (c + 1) * BCORE],
                        in_=ps[:, :Nk].rearrange("p (t b) -> p t b", t=Tb),
                        func=Act.Copy)

            def steps(k):
                """Per-step fused recurrences for iteration k:
                L0 on block k (if k < NB), L1 on block k-2 (if k >= 2)."""
                l0 = k if k < NB else None
                l1 = k - 2 if k >= 2 else None
                n0 = BLOCK_SIZES[l0] if l0 is not None else 0
                n1 = BLOCK_SIZES[l1] if l1 is not None else 0
                drv = drv9 if k == NB + 1 else drv_t[k]
                if l0 is not None:
                    s0blk = s0_pool.tile([128, TBM * 512], f16, tag="s0")
                    prev_blk = s0_t.get(l0 - 1)
                    s0_t[l0] = s0blk
                for t in range(max(n0, n1)):
                    do0 = l0 is not None and t < n0
                    do1 = l1 is not None and t < n1
                    p = gstep[0] % 2
                    gstep[0] += 1
                    src, dst = nm[p], nm[1 - p]
                    slot = drv[:, t * 512:(t + 1) * 512]
                    if do0 and do1:
                        ri = nc.vector._custom_dve(
                            OP_RESET, out=dst[:], in0=src[:],
                            in1=slot, s0=BETA, s1=-THR)
                    elif do0:
                        ri = nc.vector._custom_dve(
                            OP_RESET, out=dst[:, 0:256], in0=src[:, 0:256],
                            in1=slot[:, 0:256], s0=BETA, s1=-THR)
                    elif do1:
                        ri = nc.vector._custom_dve(
                            OP_RESET, out=dst[:, 256:512], in0=src[:, 256:512],
                            in1=slot[:, 256:512], s0=BETA, s1=-THR)
                    ri.ins.perf_max = 1
                    if do0 and do1:
                        # trace-EMA + weighted-spike accumulation as two 2x
                        # ops on the [trace | abar] halves of the fp16 slot
                        if t > 0:
                            tb_ = (t - 1) * 512
                            pb = s0blk
                        else:
                            tb_ = (BLOCK_SIZES[l0 - 1] - 1) * 512
                            pb = prev_blk
                        ti = nc.vector._custom_dve(
                            OP_TRACE, out=s0blk[:, t * 512:t * 512 + 256],
                            in0=dst[:, 0:256], in1=pb[:, tb_:tb_ + 256],
                            s0=ALPHA)
                        ti.ins.perf_max = 1
                        ai = nc.vector._custom_dve(
                            OP_ABAR,
                            out=s0blk[:, t * 512 + 256:(t + 1) * 512],
                            in0=dst[:, 256:512],
                            in1=pb[:, tb_ + 256:tb_ + 512],
                            s0=float(W[TSTART[l1] + t]))
                        ai.ins.perf_max = 1
                    elif do1:
                        # abar-only step: hand the running value off from
                        # the last combined slot to the fp32 abar tile.
                        # Blocks NB-2 and NB-1 accumulate into abar2 so
                        # abar (blocks 0..NB-3) is final one iteration
                        # earlier and its W2 matmul runs warm.
                        ab = abar2 if l1 >= NB - 2 else abar
                        if (l0 is not None and t == n0 and ab is abar):
                            ab_in = s0blk[:, (t - 1) * 512 + 256:t * 512]
                        else:
                            ab_in = ab[:]
                        ai = nc.vector._custom_dve(
                            OP_ABAR, out=ab[:], in0=dst[:, 256:512],
                            in1=ab_in, s0=float(W[TSTART[l1] + t]))
                        ai.ins.perf_max = 1
                    elif do0:
                        # trace-only step (layer-1 not yet in flight)
                        tslot = s0blk[:, t * 512:t * 512 + 256]
                        if t > 0:
                            tprev = s0blk[:, (t - 1) * 512:(t - 1) * 512 + 256]
                        elif prev_blk is not None:
                            pt = BLOCK_SIZES[l0 - 1] - 1
                            tprev = prev_blk[:, pt * 512:pt * 512 + 256]
                        else:
                            tprev = None
                        if tprev is None:
                            nc.vector.tensor_scalar(
                                out=tslot, in0=dst[:, 0:256], scalar1=0.0,
                                scalar2=None, op0=A.is_equal)
                        else:
                            ti = nc.vector._custom_dve(
                                OP_TRACE, out=tslot, in0=dst[:, 0:256],
                                in1=tprev, s0=ALPHA)
                            ti.ins.perf_max = 1
                    if do1 and not do0 and t % 3 == 1:
                        # keep-warm: the L1-only drain steps leave the PE
                        # idle past the HAM window; a junk matmul pinned to
                        # this step's membrane tile keeps the clock at 8/8
                        nc.tensor.matmul(ps_w[:], lhsT=junkf[:, :128],
                                         rhs=dst[:, 256:512],
                                         start=True, stop=True)

            # ---------------- schedule ----------------
            # PE warm-up: junk matmuls with no DMA deps run during the
            # input-DMA fill, so the HAM un-throttles (K=8/8) before the
            # first real matmul instead of ~6us after it
            for _ in range(26):
                nc.tensor.matmul(ps_w[:], lhsT=junk[:, :128],
                                 rhs=junk[:, 128:384], start=True, stop=True)
            # fp16 W0 first (it gates the first mm0), split into 4 chunk
            # DMAs so the transfers spread across queues and the first
            # matmuls can start on the earliest chunk
            stage_dma_rt(0)
            nc.sync.dma_start(
                out=w0h_sb[:].rearrange("p (k m) -> p k m", k=4),
                in_=W0h.rearrange("(k p) m -> p k m", p=128))
            nc.sync.dma_start(
                out=xt_sb[:].rearrange("p (c b) -> p c b", c=4),
                in_=xT.rearrange("(c p) b -> p c b", p=128))
            nc.sync.dma_start(out=b0_sb[:], in_=b0d)
            stage_dma_rt(1)
            stage_sg(0)
            nc.sync.dma_start(
                out=w1_sb[:].rearrange("p (k m) -> p k m", k=8),
                in_=W1d.rearrange("(k p) m -> p k m", p=128))
            stage_dma_rt(2)
            stage_sg(1)
            drv_t[0] = drv_pool.tile([128, 512 * TBM], f16, tag="drv",
                                     name="drv0")
            stage_mm0(0)

            for k in range(NB + 2):
                if k + 3 < NB:
                    stage_dma_rt(k + 3)
                # drive tile for iteration k+1 gets h1(k-1) and h0(k+1)
                if k + 1 <= NB:
                    drv_t[k + 1] = drv_pool.tile(
                        [128, 512 * TBM], f16, tag="drv", name=f"drv{k + 1}")
                if 1 <= k <= NB - 1:
                    stage_mm1(k - 1)
                if k + 1 < NB:
                    stage_mm0(k + 1)
                if k == NB - 1:
                    # W2 (fp16) arrives late, into a freed spike-block buffer
                    w2_sb = s_pool.tile([128, 8 * 512], f16, tag="sblk",
                                        name="w2_sb")
                    nc.sync.dma_start(
                        out=w2_sb[:].rearrange("p (k m) -> p k m", k=8),
                        in_=W2d.rearrange("(k p) m -> p k m", p=128))
                # abar-in-slot chain stitches at block-size mismatches:
                if k == 2:
                    # zero the abar lanes the first combined step will read
                    ls = BLOCK_SIZES[1] - 1
                    nc.vector.memset(
                        s0_t[1][:, ls * 512 + 256:(ls + 1) * 512], 0.0)
                if k == 3 and BLOCK_SIZES[0] < BLOCK_SIZES[2]:
                    # iter-2's combined phase ended at slot n1-1; move the
                    # running abar to the slot iter-3's t=0 will read
                    sa = BLOCK_SIZES[0] - 1
                    da = BLOCK_SIZES[2] - 1
                    nc.vector.tensor_copy(
                        s0_t[2][:, da * 512 + 256:(da + 1) * 512],
                        s0_t[2][:, sa * 512 + 256:(sa + 1) * 512])
                if k == NB - 1:
                    # iter-6 finished abar on the fp32 tile; seed it back
                    # into the slot iter-7's combined t=0 will read
                    ls = BLOCK_SIZES[NB - 2] - 1
                    nc.vector.tensor_copy(
                        s0_t[NB - 2][:, ls * 512 + 256:(ls + 1) * 512],
                        abar[:])
                if k == NB:
                    # abar (blocks 0..NB-3) went final at the end of the
                    # previous iteration: start mem2 = abar @ W2 in PSUM
                    # now, while this iteration's drain steps run, so the
                    # matmuls overlap DVE work and run warm
                    psf = ps_pool.tile([BCORE, 512], f32, tag="psf")
                    for ki in range(8):
                        nc.tensor.matmul(
                            psf[:],
                            lhsT=abar[:, ki * BCORE:(ki + 1) * BCORE],
                            rhs=w2_sb[:, ki * 512:(ki + 1) * 512],
                            start=(ki == 0), stop=False)
                steps(k)
                if k == NB - 1:
                    # last block's traces are complete 3 steps into this
                    # iteration: run its mm1 here, under ~4us of PE slack,
                    # into the dedicated tail-drive tile
                    stage_mm1(NB - 1, out_drv=drv9)
                if k + 2 < NB:
                    stage_sg(k + 2)

            # ---- final: mem2 += abar2 @ W2 (PSUM accumulate) ----
            for ki in range(8):
                nc.tensor.matmul(
                    psf[:],
                    lhsT=abar2[:, ki * BCORE:(ki + 1) * BCORE],
                    rhs=w2_sb[:, ki * 512:(ki + 1) * 512],
                    start=False, stop=(ki == 7))
            outsb = cpool.tile([BCORE, 512], f32, tag="outsb")
            nc.scalar.activation(out=outsb[:], in_=psf[:], func=Act.Copy)
            nc.sync.dma_start(out=outd, in_=outsb[:])

    nc.compile()
    _CACHE["nc"] = nc
    return nc


def make_in_maps(inputs, W0, W1, W2, random_distribution):
    inputs = np.ascontiguousarray(np.asarray(inputs, np.float32))
    W0 = np.asarray(W0, np.float32)
    W1 = np.asarray(W1, np.float32)
    W2 = np.asarray(W2, np.float32)
    R = np.asarray(random_distribution, np.float32)

    W0h16 = np.ascontiguousarray(W0[:512].astype(np.float16))
    W1r = np.ascontiguousarray(W1.astype(np.float16))
    W2r = np.ascontiguousarray(W2.astype(np.float16))
    b0 = np.ascontiguousarray(W0[512].reshape(8, 128).T)  # [128, 8]

    in_maps = []
    for i in range(NCORES):
        sl = slice(i * BCORE, (i + 1) * BCORE)
        xTi = np.ascontiguousarray(inputs[sl].T)  # [512, 32]
        RTi = np.ascontiguousarray(
            R[1:, sl, :512].transpose(2, 0, 1).reshape(512, T * BCORE))
        in_maps.append({
            "RT": RTi, "xT": xTi, "W0h": W0h16,
            "W1d": W1r, "W2d": W2r, "b0d": b0,
        })
    return in_maps


def kernel(inputs, W0, W1, W2, random_distribution):
    from concourse.bass_utils import run_bass_kernel_spmd
    nc = build_program()
    in_maps = make_in_maps(inputs, W0, W1, W2, random_distribution)
    res = run_bass_kernel_spmd(nc, in_maps, core_ids=list(range(NCORES)))
    outs = [np.asarray(res.results[i]["out"], np.float32) for i in range(NCORES)]
    return np.concatenate(outs, axis=0)


if __name__ == "__main__":
    d = np.load("/tmp/snn_inputs.npz")
    out = kernel(d["inputs"], d["W0"], d["W1"], d["W2"], d["random_distribution"])
    exp = d["expected"]
    rel = np.linalg.norm(out - exp) / np.linalg.norm(exp)
    print("kernel vs reference rel_l2:", rel)



# revision 28
# speedup vs baseline: 1.0526x; 1.0526x over previous
"""Trainium2 Bass kernel for the 3-layer spiking neural network (DSNN).

Strategy
--------
Data-parallel over batch: 256 rows / 8 cores = 32 per core, weights
replicated, zero collectives. Inside each core:

  - Layer 2 has no reset, so mem2 = (sum_t w_t out1(t)) @ W2 exactly
    (closed-form alpha/beta decay weights). The sum is split: blocks
    0..NB-3 accumulate into `abar` (final one iteration before the
    drain, so its 8 matmuls overlap the drain steps and run warm) and
    the last two blocks into `abar2` (one 8-matmul PSUM-accumulated
    pass at the very end).
  - The layer-1 synapse recurrence is folded into the matmul operand:
    mm1's moving tensor is the spike TRACE strace_t = a*strace_{t-1}
    + s0_t, so y1_t = strace_t @ W1 exactly (linearity).
  - Timestep-blocked matmuls (Tb=16 -> 512 moving columns), weights /
    spikes / trace all fp16. The PE streams at 1 col/cycle (2.0 GHz
    under the sustained-load P0 power state) and is the roofline:
    ~96 N=512 matmuls per 16-step block, LDWEIGHTS fully hidden.
  - ALL recurrent state is fp16 (membranes, drive slots, trace, abar)
    so the per-step DVE ops run in the hardware's 2X_1PORT perf mode
    via HAND-WRITTEN dual-lane uop programs (the stock custom-DVE
    lowering only emits 1x): SNN_RESETX does both layers' membrane
    decay+reset in one [128,512] op (~420ns vs ~828 at 1x/f32) using
    the select-free form out = x*(x >= -thr), x = beta*negm - h
    (0.0 encodes "spiked"); SNN_TRACEX/SNN_ABARX update the trace EMA
    and the w_t-weighted spike sum on the slot halves (~290ns each).
    The 2x programs are installed by seeding dve_ops._COMPILE_CACHE
    with a DveOpSpec carrying uops_2x + perf_max=1 and setting
    perf_max on each emitted instruction. Host-side bit-accurate
    simulation predicted rel-l2 1.3326e-2 (HW: 1.3320e-2) vs the
    2e-2 gate.
  - Layer-1 trails layer-0 by two blocks so its drive h1 = trace @ W1
    is ready; membrane state ping-pongs between nmA/nmB.
  - HAM keep-warm: junk matmuls bridge the input-DMA fill (the PE
    clock-gate un-throttles before the first real matmul) and pinned
    dummy matmuls ride the DVE-only drain steps so the final W2
    matmuls run at full clock. W0h is DMA'd first (it gates mm0) in 4
    chunk transfers; each rt block is one DMA trigger.

Per main step the DVE runs three 2x ops: RESETX [128,512] + TRACEX +
ABARX [128,256] each (~1.0us/step vs 2.65us in the original kernel).
Steady state is PE-bound at ~1.55us/step. Measured end-to-end
(traced): 163-167us, from 225.3us at session start and 273.6us for
the original unblocked kernel.
"""

import numpy as np

ALPHA = 0.9
BETA = 0.85
THR = 1.0
T = 99            # timesteps actually simulated (t = 1..99 of 100)
BCORE = 32        # batch per core
NCORES = 8
TBM = 16          # main block size (Nk = 512 moving cols)
BLOCK_SIZES = [8, 16, 16, 16, 16, 16, 8, 3]
assert sum(BLOCK_SIZES) == T
NB = len(BLOCK_SIZES)
TSTART = [sum(BLOCK_SIZES[:i]) for i in range(NB)]

_CACHE = {}


def _register_custom_ops():
    """SNN_RESET: m = in0*s0 + in1; out = m>s1 ? 0 : -m   (negated membrane;
    0.0 encodes "spiked").
    SNN_TRACE: out = in1*s0 + (in0 == 0)                  (spike trace EMA).
    SNN_ABAR:  out = (in0 == 0) ? in1 + s0 : in1          (weighted spikes).
    """
    import concourse.dve_ops as dve_ops
    if "SNN_RESET" in dve_ops._SUB_OPCODE_FOR_NAME:
        return (next(o for o in dve_ops.OPS if o.name == "SNN_RESETX"),
                next(o for o in dve_ops.OPS if o.name == "SNN_TRACEX"),
                next(o for o in dve_ops.OPS if o.name == "SNN_ABARX"),
                next(o for o in dve_ops.OPS if o.name == "SNN_TA"))
    from concourse.dve_spec import (
        Spec, Src0, Src1, Zero, select, eq, lower, _has_src1, SubIdx)
    from concourse.dve_uop import DveOpSpec

    def make(name, spec, subdim=False):
        row = dve_ops._CUSTOM_DVE_ROW_BASE + len(dve_ops.OPS)
        assert row < 0x20
        dve_ops._SUB_OPCODE_FOR_NAME[name] = row
        shas = {}
        for ver in ("v3", "v4"):
            uops = lower(spec, ver=ver)
            shas[ver] = DveOpSpec(name=name, opcode=row, uops=uops,
                                  rd1_en=_has_src1(spec)).sha(ver)
        op = dve_ops.DveOp(name, spec, subdim=subdim, uops_sha=shas)
        dve_ops.OPS.append(op)
        dve_ops.CUSTOM_DVE_SPECS[name] = spec
        return op

    from concourse.dve_spec import C0, C1
    f32 = np.float32
    _m = Src0 * C0 + Src1
    reset = make("SNN_RESET", Spec(
        body=select(_m > C1, Zero, Zero - _m),
        reference=lambda in0, in1, s0, s1, imm2:
            np.where((in0 * f32(s0) + in1) > f32(s1),
                     f32(0.0), -(in0 * f32(s0) + in1)).astype(f32),
    ))
    trace = make("SNN_TRACE", Spec(
        body=Src1 * C0 + eq(Src0, Zero),
        reference=lambda in0, in1, s0, s1, imm2:
            (in1 * f32(s0) + (in0 == 0.0)).astype(f32),
    ))
    abar_op = make("SNN_ABAR", Spec(
        body=select(eq(Src0, Zero), Src1 + C0, Src1),
        reference=lambda in0, in1, s0, s1, imm2:
            np.where(in0 == 0.0, in1 + f32(s0), in1).astype(f32),
    ))

    def _ta_ref(in0, in1, s0, s1, imm2):
        # in0/in1/out: [P, 2, N] pages; page 0 = trace, page 1 = abar
        sp = (in0 == 0.0).astype(f32)
        out = np.empty_like(in1, dtype=f32)
        out[:, 0] = in1[:, 0] * f32(s0) + sp[:, 0]
        out[:, 1] = np.where(in0[:, 1] == 0.0, in1[:, 1] + f32(s1), in1[:, 1])
        return out

    _sp = eq(Src0, Zero)
    ta_op = make("SNN_TA", Spec(
        body=select(SubIdx, Src1 + _sp * C1, Src1 * C0 + _sp),
        reference=_ta_ref,
    ), subdim=True)

    # ---- SNN_RESETX: fp16 membrane update with a hand-built 2X_1PORT
    # program. Same semantics as SNN_RESET but on the negated form
    # x = beta*negm - h (so s0=+beta, s1=-thr):
    #   out = x * (x >= -thr)   (spiked -> exact 0.0)
    # With all-fp16 step-1 operands and perf_max=1 the engine runs the 2x
    # program: 2 elems/cycle, halving the dominant per-step DVE cost.
    from concourse.dve_uop import (
        UopConfig, InpSel, AluInp, DelayInp, OutSel, OutPath,
        AluOp as UAluOp, Trigger, ENABLE)

    _x = Src0 * C0 - Src1
    rx_spec = Spec(
        body=_x * (_x >= C1),
        reference=lambda in0, in1, s0, s1, imm2: (
            lambda xv: np.where(xv >= f32(s1), xv, f32(0.0)).astype(f32)
        )(in0 * f32(s0) - in1),
    )
    rx_row = dve_ops._CUSTOM_DVE_ROW_BASE + len(dve_ops.OPS)
    assert rx_row < 0x20
    dve_ops._SUB_OPCODE_FOR_NAME["SNN_RESETX"] = rx_row
    rx_1x = lower(rx_spec, ver="v3")
    assert len(rx_1x) == 1

    # lanes: 0=SRC_0(negm lo) 1=SRC_1(h lo) 2=SRC_0_HI 3=SRC_1_HI
    #        4=C0(beta) 5=C1(-thr); chains at b0: d[c] <- lane c+1
    u = UopConfig()
    u.enable_input(InpSel.SRC_0, 0)
    u.enable_input(InpSel.SRC_1, 1)
    u.enable_input(InpSel.SRC_0_HI, 2)
    u.enable_input(InpSel.SRC_1_HI, 3)
    u.enable_input(InpSel.CONST_0, 4)
    u.enable_input(InpSel.CONST_1, 5)
    u.require_inp0 = ENABLE
    u.require_inp1 = ENABLE
    u.trigger = (Trigger.SRC_TENSOR_DONE, Trigger.NONE, Trigger.NONE)
    u.enable_output(OutSel.DELAY_0, OutPath.WR0_LO)
    u.enable_output(OutSel.ALU_OUT, OutPath.WR0_HI)
    b = u.datapath_config
    b[0].enable_alu(UAluOp.MULTIPLY, AluInp.PREV_ALU_OUT, AluInp.PREV_DELAY_3)
    for c in range(5):
        b[0].enable_delay_from_src(DelayInp.PREV_DELAY, c)
    b[1].enable_alu(UAluOp.SUBTRACT, AluInp.PREV_ALU_OUT, AluInp.PREV_DELAY_0)
    for c in (1, 2, 3, 4):
        b[1].pass_through_delay(c)
    b[2].enable_alu(UAluOp.MULTIPLY, AluInp.PREV_DELAY_1, AluInp.PREV_DELAY_3)
    b[2].enable_delay_from_src(DelayInp.PREV_ALU_OUT, 0)
    for c in (2, 4):
        b[2].pass_through_delay(c)
    b[3].enable_alu(UAluOp.SUBTRACT, AluInp.PREV_ALU_OUT, AluInp.PREV_DELAY_2)
    for c in (0, 4):
        b[3].pass_through_delay(c)
    b[4].enable_alu(UAluOp.IS_GE, AluInp.PREV_DELAY_0, AluInp.PREV_DELAY_4)
    b[4].enable_delay_from_src(DelayInp.PREV_ALU_OUT, 1)
    for c in (0, 4):
        b[4].pass_through_delay(c)
    b[5].enable_alu(UAluOp.MULTIPLY, AluInp.PREV_ALU_OUT, AluInp.PREV_DELAY_0)
    for c in (1, 4):
        b[5].pass_through_delay(c)
    b[6].enable_alu(UAluOp.IS_GE, AluInp.PREV_DELAY_1, AluInp.PREV_DELAY_4)
    b[6].enable_delay_from_src(DelayInp.PREV_ALU_OUT, 0)
    b[6].pass_through_delay(1)
    b[7].enable_alu(UAluOp.MULTIPLY, AluInp.PREV_ALU_OUT, AluInp.PREV_DELAY_1)
    b[7].pass_through_delay(0)
    u.validate("v3")

    rx_spec2 = DveOpSpec(name="SNN_RESETX", opcode=rx_row, uops=rx_1x,
                         uops_2x=[u], perf_max=1, rd1_en=True)
    dve_ops._COMPILE_CACHE[("SNN_RESETX", "v3")] = rx_spec2
    rx_shas = {"v3": rx_spec2.sha("v3"),
               "v4": DveOpSpec(name="SNN_RESETX", opcode=rx_row,
                               uops=lower(rx_spec, ver="v4"),
                               rd1_en=True).sha("v4")}
    resetx = dve_ops.DveOp("SNN_RESETX", rx_spec, subdim=False,
                           uops_sha=rx_shas)
    dve_ops.OPS.append(resetx)
    dve_ops.CUSTOM_DVE_SPECS["SNN_RESETX"] = rx_spec

    # ---- SNN_TRACEX / SNN_ABARX: 2x variants of the trace-EMA and the
    # weighted-spike accumulation. Both bodies are 3 ALU ops per lane, so
    # the duplicated (lo|hi) chain fits in 6 of the 8 datapath blocks.
    def _make_2x(name, spec, wire):
        row2 = dve_ops._CUSTOM_DVE_ROW_BASE + len(dve_ops.OPS)
        assert row2 < 0x20
        dve_ops._SUB_OPCODE_FOR_NAME[name] = row2
        u1x = lower(spec, ver="v3")
        assert len(u1x) == 1
        u2 = UopConfig()
        u2.enable_input(InpSel.SRC_0, 0)
        u2.enable_input(InpSel.SRC_1, 1)
        u2.enable_input(InpSel.SRC_0_HI, 2)
        u2.enable_input(InpSel.SRC_1_HI, 3)
        u2.enable_input(InpSel.CONST_0, 4)
        u2.enable_input(InpSel.ZERO, 5)
        u2.require_inp0 = ENABLE
        u2.require_inp1 = ENABLE
        u2.trigger = (Trigger.SRC_TENSOR_DONE, Trigger.NONE, Trigger.NONE)
        u2.enable_output(OutSel.DELAY_0, OutPath.WR0_LO)
        u2.enable_output(OutSel.ALU_OUT, OutPath.WR0_HI)
        wire(u2.datapath_config)
        u2.validate("v3")
        spec2 = DveOpSpec(name=name, opcode=row2, uops=u1x, uops_2x=[u2],
                          perf_max=1, rd1_en=True)
        dve_ops._COMPILE_CACHE[(name, "v3")] = spec2
        shas2 = {"v3": spec2.sha("v3"),
                 "v4": DveOpSpec(name=name, opcode=row2,
                                 uops=lower(spec, ver="v4"),
                                 rd1_en=True).sha("v4")}
        op2 = dve_ops.DveOp(name, spec, subdim=False, uops_sha=shas2)
        dve_ops.OPS.append(op2)
        dve_ops.CUSTOM_DVE_SPECS[name] = spec
        return op2

    # lanes: 0=SRC_0 1=SRC_1 2=SRC_0_HI 3=SRC_1_HI 4=C0 5=ZERO
    # chains at b0: d0<-L1 d1<-L2 d2<-L3 d3<-L4 d4<-L5
    def _wire_trace(b):
        # out = Src1*C0 + (Src0 == 0)
        b[0].enable_alu(UAluOp.IS_EQ, AluInp.PREV_ALU_OUT, AluInp.PREV_DELAY_4)
        for c in range(5):
            b[0].enable_delay_from_src(DelayInp.PREV_DELAY, c)
        b[1].enable_alu(UAluOp.MULTIPLY, AluInp.PREV_DELAY_0, AluInp.PREV_DELAY_3)
        b[1].enable_delay_from_src(DelayInp.PREV_ALU_OUT, 0)
        for c in (1, 2, 3, 4):
            b[1].pass_through_delay(c)
        b[2].enable_alu(UAluOp.ADD, AluInp.PREV_ALU_OUT, AluInp.PREV_DELAY_0)
        for c in (1, 2, 3, 4):
            b[2].pass_through_delay(c)
        b[3].enable_alu(UAluOp.IS_EQ, AluInp.PREV_DELAY_1, AluInp.PREV_DELAY_4)
        b[3].enable_delay_from_src(DelayInp.PREV_ALU_OUT, 0)
        for c in (2, 3):
            b[3].pass_through_delay(c)
        b[4].enable_alu(UAluOp.MULTIPLY, AluInp.PREV_DELAY_2, AluInp.PREV_DELAY_3)
        b[4].enable_delay_from_src(DelayInp.PREV_ALU_OUT, 1)
        b[4].pass_through_delay(0)
        b[5].enable_alu(UAluOp.ADD, AluInp.PREV_ALU_OUT, AluInp.PREV_DELAY_1)
        b[5].pass_through_delay(0)
        b[6].pass_through_alu()
        b[6].pass_through_delay(0)
        b[7].pass_through_alu()
        b[7].pass_through_delay(0)

    def _wire_abar(b):
        # out = Src1 + (Src0 == 0)*C0
        b[0].enable_alu(UAluOp.IS_EQ, AluInp.PREV_ALU_OUT, AluInp.PREV_DELAY_4)
        for c in range(5):
            b[0].enable_delay_from_src(DelayInp.PREV_DELAY, c)
        b[1].enable_alu(UAluOp.MULTIPLY, AluInp.PREV_ALU_OUT, AluInp.PREV_DELAY_3)
        for c in (0, 1, 2, 3, 4):
            b[1].pass_through_delay(c)
        b[2].enable_alu(UAluOp.ADD, AluInp.PREV_ALU_OUT, AluInp.PREV_DELAY_0)
        for c in (1, 2, 3, 4):
            b[2].pass_through_delay(c)
        b[3].enable_alu(UAluOp.IS_EQ, AluInp.PREV_DELAY_1, AluInp.PREV_DELAY_4)
        b[3].enable_delay_from_src(DelayInp.PREV_ALU_OUT, 0)
        for c in (2, 3):
            b[3].pass_through_delay(c)
        b[4].enable_alu(UAluOp.MULTIPLY, AluInp.PREV_ALU_OUT, AluInp.PREV_DELAY_3)
        for c in (0, 2):
            b[4].pass_through_delay(c)
        b[5].enable_alu(UAluOp.ADD, AluInp.PREV_ALU_OUT, AluInp.PREV_DELAY_2)
        b[5].pass_through_delay(0)
        b[6].pass_through_alu()
        b[6].pass_through_delay(0)
        b[7].pass_through_alu()
        b[7].pass_through_delay(0)

    tracex = _make_2x("SNN_TRACEX", Spec(
        body=Src1 * C0 + eq(Src0, Zero),
        reference=lambda in0, in1, s0, s1, imm2:
            (in1 * f32(s0) + (in0 == 0.0)).astype(f32)), _wire_trace)
    abarx = _make_2x("SNN_ABARX", Spec(
        body=select(eq(Src0, Zero), Src1 + C0, Src1),
        reference=lambda in0, in1, s0, s1, imm2:
            np.where(in0 == 0.0, in1 + f32(s0), in1).astype(f32)), _wire_abar)

    return resetx, tracex, abarx, ta_op


def _round_m11(x):
    # hw float32r = e8m11, round-to-nearest on the 12 dropped bits
    xi = np.ascontiguousarray(np.asarray(x, np.float32)).view(np.uint32).astype(np.uint64)
    bias = np.uint64(0x7FF) + ((xi >> np.uint64(12)) & np.uint64(1))
    return ((xi + bias) & np.uint64(0xFFFFF000)).astype(np.uint32).view(np.float32)


def _decay_weights():
    # w_j = sum_{k=0}^{T-1-j} BETA^(T-1-j-k) * ALPHA^k
    w = np.zeros(T, np.float64)
    for j in range(T):
        n = T - 1 - j
        k = np.arange(n + 1)
        w[j] = np.sum(BETA ** (n - k) * (ALPHA ** k))
    return w.astype(np.float32)


def build_program():
    if "nc" in _CACHE:
        return _CACHE["nc"]
    import concourse.bacc as bacc
    import concourse.mybir as mybir
    import concourse.tile as tile

    f32 = mybir.dt.float32
    f32r = mybir.dt.float32r
    f16 = mybir.dt.float16
    A = mybir.AluOpType
    Act = mybir.ActivationFunctionType

    OP_RESET, OP_TRACE, OP_ABAR, OP_TA = _register_custom_ops()
    W = _decay_weights()

    nc = bacc.Bacc("TRN2", target_bir_lowering=False, debug=False,
                   enable_asserts=False, num_devices=NCORES)

    RT = nc.dram_tensor("RT", [512, T * BCORE], f32, kind="ExternalInput").ap()
    xT = nc.dram_tensor("xT", [512, BCORE], f32, kind="ExternalInput").ap()
    W0h = nc.dram_tensor("W0h", [512, 1024], f16, kind="ExternalInput").ap()
    W1d = nc.dram_tensor("W1d", [1024, 1024], f16, kind="ExternalInput").ap()
    W2d = nc.dram_tensor("W2d", [1024, 512], f16, kind="ExternalInput").ap()
    b0d = nc.dram_tensor("b0d", [128, 8], f32, kind="ExternalInput").ap()
    outd = nc.dram_tensor("out", [BCORE, 512], f32, kind="ExternalOutput").ap()

    with tile.TileContext(nc) as tc:
        with (
            tc.tile_pool(name="const", bufs=1) as cpool,
            tc.tile_pool(name="rt", bufs=3) as rt_pool,
            tc.tile_pool(name="sblk", bufs=2) as s_pool,
            tc.tile_pool(name="s0p", bufs=2) as s0_pool,
            tc.tile_pool(name="drv", bufs=2) as drv_pool,
            tc.tile_pool(name="ps", bufs=3, space="PSUM") as ps_pool,
            tc.tile_pool(name="warm", bufs=1, space="PSUM") as warm_pool,
        ):
            # ---- constants ----
            w0h_sb = cpool.tile([128, 4 * 1024], f16, tag="w0h")
            w1_sb = cpool.tile([128, 8 * 1024], f16, tag="w1")
            b0_sb = cpool.tile([128, 8], f32, tag="b0")
            xt_sb = cpool.tile([128, 4 * BCORE], f32, tag="xt")

            # PE warm-up fodder: junk operands with no DMA deps, plus a
            # stable fp32 tile the drain keep-warm matmuls stream from.
            junk = cpool.tile([128, 384], f16, tag="junk")
            junkf = cpool.tile([128, 256], f16, tag="junkf")
            nc.gpsimd.memset(junk[:], 0.0)
            nc.gpsimd.memset(junkf[:], 0.0)
            ps_w = warm_pool.tile([128, 256], f32, tag="psw")

            # ---- state ----
            # negm ping-pong: [0:256) = layer-0 negm (c,b), [256:512) = layer-1
            nmA = cpool.tile([128, 512], f16, tag="nmA")
            nmB = cpool.tile([128, 512], f16, tag="nmB")
            abar = cpool.tile([128, 256], f16, tag="abar")
            abar2 = cpool.tile([128, 256], f16, tag="abar2")
            drv9 = cpool.tile([128, BLOCK_SIZES[NB - 1] * 512], f16,
                              tag="drv9")
            for st in (nmA, nmB, abar, abar2):
                nc.vector.memset(st[:], 0.0)
            nm = [nmA, nmB]
            gstep = [0]

            rt4 = RT.rearrange("(c p) n -> p c n", p=128)
            rt_t, sblk_t, s0_t, drv_t = {}, {}, {}, {}

            def stage_dma_rt(k):
                Tb = BLOCK_SIZES[k]
                Nk = Tb * BCORE
                rt = rt_pool.tile([128, 4 * TBM * BCORE], f32, tag="rt")
                nc.sync.dma_start(
                    out=rt[:, :4 * Nk].rearrange("p (c n) -> p c n", c=4),
                    in_=rt4[:, :, TSTART[k] * BCORE: TSTART[k] * BCORE + Nk])
                rt_t[k] = rt

            def stage_sg(k):
                # spike-gen: compare x (broadcast over t) against rt.
                # Output dtype matches the W0 flavor mm0 will use:
                # f32r for early blocks, fp16 for the small late blocks.
                Tb = BLOCK_SIZES[k]
                Nk = Tb * BCORE
                rt = rt_t.pop(k)
                sblk = s_pool.tile([128, 4 * TBM * BCORE], f16, tag="sblk")
                xc = (xt_sb[:].rearrange("p (c b) -> p c b", c=4)
                      .unsqueeze(2).broadcast_to([128, 4, Tb, BCORE]))
                ssl = sblk[:, :4 * Nk].rearrange("p (c t b) -> p c t b", c=4, t=Tb)
                rsl = rt[:, :4 * Nk].rearrange("p (c t b) -> p c t b", c=4, t=Tb)
                if k == 0:
                    # chunked so the first compares pipeline with the DMA
                    for c in range(4):
                        nc.vector.tensor_tensor(
                            out=ssl[:, c:c + 1], in0=xc[:, c:c + 1],
                            in1=rsl[:, c:c + 1], op=A.is_gt)
                else:
                    nc.vector.tensor_tensor(out=ssl, in0=xc, in1=rsl, op=A.is_gt)
                sblk_t[k] = sblk

            def stage_mm0(k):
                # H0 = S @ W0 -> drive tile k, slot lanes [0:256), t-major
                Tb = BLOCK_SIZES[k]
                Nk = Tb * BCORE
                sblk = sblk_t.pop(k)
                w0t = w0h_sb
                drv = drv_t[k]
                dv = drv[:].rearrange("p (t l) -> p t l", t=TBM)
                for c in range(8):
                    ps = ps_pool.tile([128, TBM * BCORE], f32, tag="ps")
                    for ki in range(4):
                        nc.tensor.matmul(
                            ps[:, :Nk],
                            lhsT=w0t[:, ki * 1024 + c * 128: ki * 1024 + (c + 1) * 128],
                            rhs=sblk[:, ki * Nk:(ki + 1) * Nk],
                            start=(ki == 0), stop=(ki == 3))
                    # PSUM (t,b) -> drive slots, bias fold
                    nc.scalar.activation(
                        out=dv[:, 0:Tb, c * BCORE:(c + 1) * BCORE],
                        in_=ps[:, :Nk].rearrange("p (t b) -> p t b", t=Tb),
                        func=Act.Identity, bias=b0_sb[:, c:c + 1], scale=1.0)

            def stage_mm1(k, out_drv=None):
                # H1 = strace @ W1 -> drive tile k+2, slot lanes [256:512)
                Tb = BLOCK_SIZES[k]
                Nk = Tb * BCORE
                s0blk = s0_t[k]
                # slots are 512 wide: lanes [0:256) trace, [256:512) abar
                s0v = s0blk[:, :Tb * 512].rearrange("p (t l) -> p t l", t=Tb)
                if out_drv is not None:
                    drv, tdim = out_drv, Tb
                else:
                    drv, tdim = drv_t[k + 2], TBM
                dv = drv[:].rearrange("p (t l) -> p t l", t=tdim)
                for c in range(8):
                    ps = ps_pool.tile([128, TBM * BCORE], f32, tag="ps")
                    for ki in range(8):
                        nc.tensor.matmul(
                            ps[:, :Nk],
                            lhsT=w1_sb[:, ki * 1024 + c * 128: ki * 1024 + (c + 1) * 128],
                            rhs=s0v[:, :, ki * BCORE:(ki + 1) * BCORE],
                            start=(ki == 0), stop=(ki == 7))
                    nc.scalar.activation(
                        out=dv[:, 0:Tb, 256 + c * BCORE:256 + (c + 1) * BCORE],
                        in_=ps[:, :Nk].rearrange("p (t b) -> p t b", t=Tb),
                        func=Act.Copy)

            def steps(k):
                """Per-step fused recurrences for iteration k:
                L0 on block k (if k < NB), L1 on block k-2 (if k >= 2)."""
                l0 = k if k < NB else None
                l1 = k - 2 if k >= 2 else None
                n0 = BLOCK_SIZES[l0] if l0 is not None else 0
                n1 = BLOCK_SIZES[l1] if l1 is not None else 0
                drv = drv9 if k == NB + 1 else drv_t[k]
                if l0 is not None:
                    s0blk = s0_pool.tile([128, TBM * 512], f16, tag="s0")
                    prev_blk = s0_t.get(l0 - 1)
                    s0_t[l0] = s0blk
                for t in range(max(n0, n1)):
                    do0 = l0 is not None and t < n0
                    do1 = l1 is not None and t < n1
                    p = gstep[0] % 2
                    gstep[0] += 1
                    src, dst = nm[p], nm[1 - p]
                    slot = drv[:, t * 512:(t + 1) * 512]
                    if do0 and do1:
                        ri = nc.vector._custom_dve(
                            OP_RESET, out=dst[:], in0=src[:],
                            in1=slot, s0=BETA, s1=-THR)
                    elif do0:
                        ri = nc.vector._custom_dve(
                            OP_RESET, out=dst[:, 0:256], in0=src[:, 0:256],
                            in1=slot[:, 0:256], s0=BETA, s1=-THR)
                    elif do1:
                        ri = nc.vector._custom_dve(
                            OP_RESET, out=dst[:, 256:512], in0=src[:, 256:512],
                            in1=slot[:, 256:512], s0=BETA, s1=-THR)
                    ri.ins.perf_max = 1
                    if do0 and do1:
                        # trace-EMA + weighted-spike accumulation as two 2x
                        # ops on the [trace | abar] halves of the fp16 slot
                        if t > 0:
                            tb_ = (t - 1) * 512
                            pb = s0blk
                        else:
                            tb_ = (BLOCK_SIZES[l0 - 1] - 1) * 512
                            pb = prev_blk
                        ti = nc.vector._custom_dve(
                            OP_TRACE, out=s0blk[:, t * 512:t * 512 + 256],
                            in0=dst[:, 0:256], in1=pb[:, tb_:tb_ + 256],
                            s0=ALPHA)
                        ti.ins.perf_max = 1
                        ai = nc.vector._custom_dve(
                            OP_ABAR,
                            out=s0blk[:, t * 512 + 256:(t + 1) * 512],
                            in0=dst[:, 256:512],
                            in1=pb[:, tb_ + 256:tb_ + 512],
                            s0=float(W[TSTART[l1] + t]))
                        ai.ins.perf_max = 1
                    elif do1:
                        # abar-only step: hand the running value off from
                        # the last combined slot to the fp32 abar tile.
                        # Blocks NB-2 and NB-1 accumulate into abar2 so
                        # abar (blocks 0..NB-3) is final one iteration
                        # earlier and its W2 matmul runs warm.
                        ab = abar2 if l1 >= NB - 2 else abar
                        if (l0 is not None and t == n0 and ab is abar):
                            ab_in = s0blk[:, (t - 1) * 512 + 256:t * 512]
                        else:
                            ab_in = ab[:]
                        ai = nc.vector._custom_dve(
                            OP_ABAR, out=ab[:], in0=dst[:, 256:512],
                            in1=ab_in, s0=float(W[TSTART[l1] + t]))
                        ai.ins.perf_max = 1
                    elif do0:
                        # trace-only step (layer-1 not yet in flight)
                        tslot = s0blk[:, t * 512:t * 512 + 256]
                        if t > 0:
                            tprev = s0blk[:, (t - 1) * 512:(t - 1) * 512 + 256]
                        elif prev_blk is not None:
                            pt = BLOCK_SIZES[l0 - 1] - 1
                            tprev = prev_blk[:, pt * 512:pt * 512 + 256]
                        else:
                            tprev = None
                        if tprev is None:
                            nc.vector.tensor_scalar(
                                out=tslot, in0=dst[:, 0:256], scalar1=0.0,
                                scalar2=None, op0=A.is_equal)
                        else:
                            ti = nc.vector._custom_dve(
                                OP_TRACE, out=tslot, in0=dst[:, 0:256],
                                in1=tprev, s0=ALPHA)
                            ti.ins.perf_max = 1
                    if do1 and not do0 and t % 3 == 1:
                        # keep-warm: the L1-only drain steps leave the PE
                        # idle past the HAM window; a junk matmul pinned to
                        # this step's membrane tile keeps the clock at 8/8
                        nc.tensor.matmul(ps_w[:], lhsT=junkf[:, :128],
                                         rhs=dst[:, 256:512],
                                         start=True, stop=True)

            # ---------------- schedule ----------------
            # PE warm-up: junk matmuls with no DMA deps run during the
            # input-DMA fill, so the HAM un-throttles (K=8/8) before the
            # first real matmul instead of ~6us after it
            for _ in range(26):
                nc.tensor.matmul(ps_w[:], lhsT=junk[:, :128],
                                 rhs=junk[:, 128:384], start=True, stop=True)
            # fp16 W0 first (it gates the first mm0), split into 4 chunk
            # DMAs so the transfers spread across queues and the first
            # matmuls can start on the earliest chunk
            stage_dma_rt(0)
            w0r = W0h.rearrange("(k p) m -> p k m", p=128)
            for ki in range(4):
                nc.sync.dma_start(
                    out=w0h_sb[:, ki * 1024:(ki + 1) * 1024],
                    in_=w0r[:, ki])
            nc.sync.dma_start(
                out=xt_sb[:].rearrange("p (c b) -> p c b", c=4),
                in_=xT.rearrange("(c p) b -> p c b", p=128))
            nc.sync.dma_start(out=b0_sb[:], in_=b0d)
            stage_dma_rt(1)
            stage_sg(0)
            nc.sync.dma_start(
                out=w1_sb[:].rearrange("p (k m) -> p k m", k=8),
                in_=W1d.rearrange("(k p) m -> p k m", p=128))
            stage_dma_rt(2)
            stage_sg(1)
            drv_t[0] = drv_pool.tile([128, 512 * TBM], f16, tag="drv",
                                     name="drv0")
            stage_mm0(0)

            for k in range(NB + 2):
                if k + 3 < NB:
                    stage_dma_rt(k + 3)
                # drive tile for iteration k+1 gets h1(k-1) and h0(k+1)
                if k + 1 <= NB:
                    drv_t[k + 1] = drv_pool.tile(
                        [128, 512 * TBM], f16, tag="drv", name=f"drv{k + 1}")
                if 1 <= k <= NB - 1:
                    stage_mm1(k - 1)
                if k + 1 < NB:
                    stage_mm0(k + 1)
                if k == NB - 1:
                    # W2 (fp16) arrives late, into a freed spike-block buffer
                    w2_sb = s_pool.tile([128, 8 * 512], f16, tag="sblk",
                                        name="w2_sb")
                    nc.sync.dma_start(
                        out=w2_sb[:].rearrange("p (k m) -> p k m", k=8),
                        in_=W2d.rearrange("(k p) m -> p k m", p=128))
                # abar-in-slot chain stitches at block-size mismatches:
                if k == 2:
                    # zero the abar lanes the first combined step will read
                    ls = BLOCK_SIZES[1] - 1
                    nc.vector.memset(
                        s0_t[1][:, ls * 512 + 256:(ls + 1) * 512], 0.0)
                if k == 3 and BLOCK_SIZES[0] < BLOCK_SIZES[2]:
                    # iter-2's combined phase ended at slot n1-1; move the
                    # running abar to the slot iter-3's t=0 will read
                    sa = BLOCK_SIZES[0] - 1
                    da = BLOCK_SIZES[2] - 1
                    nc.vector.tensor_copy(
                        s0_t[2][:, da * 512 + 256:(da + 1) * 512],
                        s0_t[2][:, sa * 512 + 256:(sa + 1) * 512])
                if k == NB - 1:
                    # iter-6 finished abar on the fp32 tile; seed it back
                    # into the slot iter-7's combined t=0 will read
                    ls = BLOCK_SIZES[NB - 2] - 1
                    nc.vector.tensor_copy(
                        s0_t[NB - 2][:, ls * 512 + 256:(ls + 1) * 512],
                        abar[:])
                if k == NB:
                    # abar (blocks 0..NB-3) went final at the end of the
                    # previous iteration: start mem2 = abar @ W2 in PSUM
                    # now, while this iteration's drain steps run, so the
                    # matmuls overlap DVE work and run warm
                    psf = ps_pool.tile([BCORE, 512], f32, tag="psf")
                    for ki in range(8):
                        nc.tensor.matmul(
                            psf[:],
                            lhsT=abar[:, ki * BCORE:(ki + 1) * BCORE],
                            rhs=w2_sb[:, ki * 512:(ki + 1) * 512],
                            start=(ki == 0), stop=False)
                steps(k)
                if k == NB - 1:
                    # last block's traces are complete 3 steps into this
                    # iteration: run its mm1 here, under ~4us of PE slack,
                    # into the dedicated tail-drive tile
                    stage_mm1(NB - 1, out_drv=drv9)
                if k + 2 < NB:
                    stage_sg(k + 2)

            # ---- final: mem2 += abar2 @ W2 (PSUM accumulate) ----
            for ki in range(8):
                nc.tensor.matmul(
                    psf[:],
                    lhsT=abar2[:, ki * BCORE:(ki + 1) * BCORE],
                    rhs=w2_sb[:, ki * 512:(ki + 1) * 512],
                    start=False, stop=(ki == 7))
            outsb = cpool.tile([BCORE, 512], f32, tag="outsb")
            nc.scalar.activation(out=outsb[:], in_=psf[:], func=Act.Copy)
            nc.sync.dma_start(out=outd, in_=outsb[:])

    nc.compile()
    _CACHE["nc"] = nc
    return nc


def make_in_maps(inputs, W0, W1, W2, random_distribution):
    inputs = np.ascontiguousarray(np.asarray(inputs, np.float32))
    W0 = np.asarray(W0, np.float32)
    W1 = np.asarray(W1, np.float32)
    W2 = np.asarray(W2, np.float32)
    R = np.asarray(random_distribution, np.float32)

    W0h16 = np.ascontiguousarray(W0[:512].astype(np.float16))
    W1r = np.ascontiguousarray(W1.astype(np.float16))
    W2r = np.ascontiguousarray(W2.astype(np.float16))
    b0 = np.ascontiguousarray(W0[512].reshape(8, 128).T)  # [128, 8]

    in_maps = []
    for i in range(NCORES):
        sl = slice(i * BCORE, (i + 1) * BCORE)
        xTi = np.ascontiguousarray(inputs[sl].T)  # [512, 32]
        RTi = np.ascontiguousarray(
            R[1:, sl, :512].transpose(2, 0, 1).reshape(512, T * BCORE))
        in_maps.append({
            "RT": RTi, "xT": xTi, "W0h": W0h16,
            "W1d": W1r, "W2d": W2r, "b0d": b0,
        })
    return in_maps


def kernel(inputs, W0, W1, W2, random_distribution):
    from concourse.bass_utils import run_bass_kernel_spmd
    nc = build_program()
    in_maps = make_in_maps(inputs, W0, W1, W2, random_distribution)
    res = run_bass_kernel_spmd(nc, in_maps, core_ids=list(range(NCORES)))
    outs = [np.asarray(res.results[i]["out"], np.float32) for i in range(NCORES)]
    return np.concatenate(outs, axis=0)


if __name__ == "__main__":
    d = np.load("/tmp/snn_inputs.npz")
    out = kernel(d["inputs"], d["W0"], d["W1"], d["W2"], d["random_distribution"])
    exp = d["expected"]
    rel = np.linalg.norm(out - exp) / np.linalg.norm(exp)
    print("kernel vs reference rel_l2:", rel)

